# revision 51
# baseline (speedup 1.0000x reference)
"""Multi-head differential attention on 8 Trainium2 NeuronCores.

Sharding: core c -> batch c//4, head-group c%4 (4 of 16 heads).

v2 pipeline (vs baseline): the softmax exp stream on the Scalar engine is
the per-core floor (~128us), so everything is scheduled around keeping it
fed from t~14us onward:
  - prefix computes only q/k of head-pair 0; V and pair-1 q/k are emitted
    as PE filler inside pair-0's attention loop (the PE has slack while
    ACT chews exps).
  - attention inner loop is kt-pipelined: score pair (row-tiled 64x128
    auto-tiles) -> exp of [128,1024] PSUM tile -> e_sb (bf16, 16 kt tiles
    per (t,qt)) -> deferred AV (lag 2) accumulating z+denominator via the
    DH+1 ones-row.
  - softmax-normalize + GroupNorm stats + gather payload happen per
    (t,qt), and the z AllGather is split into 8 per-(t,qt) chunks that
    pipeline on the CC rings under the attention phase (the monolithic
    per-pair gathers were 2x57us, mostly exposed).
  - k-bias is dropped entirely (constant along the softmax axis), lambda
    and softmax scale are folded into Wq/bq, GroupNorm affine into Wo/bo
    on host; rstd uses exp(-0.5*ln(var+eps)) so the whole kernel needs
    one ACT table set (no mid-kernel sqrt table switch).
Each core then runs a column-parallel out-projection producing a
256-column slice of the output, assembled on host.
"""

import numpy as np
import ml_dtypes

B, S, D, H, DH = 2, 2048, 1024, 16, 64
HPC = 4            # heads per core
CW = HPC * DH      # attention columns per core (256)
EPS = 1e-5
LAMBDA_INIT = 0.8
N_CORES = 8
SCC = 16           # scalar payload columns (8 f32 as 16 bf16)
QT = 512           # q-block per (t, qt)
NQT = 4
NKT = 16
NDC = 8

_cache = {}


def _build(with_collective=True):
    from contextlib import ExitStack
    import concourse.bass as bass
    from concourse import bacc
    import concourse.tile as tile
    import concourse.mybir as mybir

    f32 = mybir.dt.float32
    bf16 = mybir.dt.bfloat16
    AF = mybir.ActivationFunctionType
    ALU = mybir.AluOpType

    nc = bacc.Bacc("TRN2", target_bir_lowering=False, debug=False,
                   num_devices=N_CORES)

    xt_d = nc.dram_tensor("xt", [D, S], bf16, kind="ExternalInput")
    wq_d = nc.dram_tensor("wq", [D, CW], bf16, kind="ExternalInput")
    wk_d = nc.dram_tensor("wk", [D, CW], bf16, kind="ExternalInput")
    wv_d = nc.dram_tensor("wv", [D, CW], bf16, kind="ExternalInput")
    wo_d = nc.dram_tensor("wo", [D, CW], bf16, kind="ExternalInput")
    # bq/bo in column layout [128, pair]: folded in as per-partition scalars
    bq_d = nc.dram_tensor("bq", [128, 2], f32, kind="ExternalInput")
    bv_d = nc.dram_tensor("bv", [CW], f32, kind="ExternalInput")
    bvf_d = nc.dram_tensor("bvf", [D], f32, kind="ExternalInput")
    bo_d = nc.dram_tensor("bo", [128, 2], f32, kind="ExternalInput")
    y_d = nc.dram_tensor("y", [2, 128, S], f32, kind="ExternalOutput")

    rs_d = nc.dram_tensor("rs_scratch", [2, 2, NQT, QT], f32)
    warm_in = nc.dram_tensor("warm_in", [1, 64], bf16)
    warm_out = nc.dram_tensor("warm_out", [4, 1, 64], bf16)
    # gather chunks: 3x 1024-col (t0qt01, t0qt23, t1qt01), then 512-col
    # t1qt2 and a final 512+SCC t1qt3 chunk carrying the GN scalars, so the
    # tail exposes only one small gather
    ag_in = nc.dram_tensor("ag_in", [3, 128, 2 * QT], bf16)
    ag_in4 = nc.dram_tensor("ag_in4", [128, QT], bf16)
    ag_in_last = nc.dram_tensor("ag_in_last", [128, QT + SCC], bf16)
    ag_out = nc.dram_tensor("ag_out", [3, 4, 128, 2 * QT], bf16)
    ag_out4 = nc.dram_tensor("ag_out4", [4, 128, QT], bf16)
    ag_out_last = nc.dram_tensor("ag_out_last", [4, 128, QT + SCC], bf16)

    GROUPS = [[0, 1, 2, 3], [4, 5, 6, 7]]

    def gather(in_ap, out_ap):
        if with_collective:
            nc.gpsimd.collective_compute(
                "AllGather", ALU.bypass, replica_groups=GROUPS,
                ins=[in_ap.opt()], outs=[out_ap.opt()])
        else:
            for g in range(4):
                nc.sync.dma_start(out=out_ap[g], in_=in_ap)

    with ExitStack() as ctx:
        tc = ctx.enter_context(tile.TileContext(nc))
        const = ctx.enter_context(tc.tile_pool(name="const", bufs=1))
        big = ctx.enter_context(tc.tile_pool(name="big", bufs=1))

        pxt = ctx.enter_context(tc.tile_pool(name="pxt", bufs=1))
        # warm-up collective input first: its gather absorbs the cross-core
        # rendezvous barrier, so it must not queue behind the bulk input DMAs
        wu_sb = pxt.tile([1, 64], bf16, tag="wu", name="wu")
        nc.vector.memset(wu_sb, 0.0)
        nc.sync.dma_start(out=warm_in[:, :], in_=wu_sb)

        # early dummy collective: absorbs the cross-core rendezvous barrier
        # under the prefix instead of delaying the first real gather
        gather(warm_in[:], warm_out[:])

        # ---- constants + x, spread over four DMA queues; q/k weights lead
        # so the prefix projections can pace with the arriving x chunks ----
        wq_sb = const.tile([128, NDC, CW], bf16, tag="wq")
        wk_sb = const.tile([128, NDC, CW], bf16, tag="wk")
        wv_sb = const.tile([128, NDC, CW], bf16, tag="wv")
        wo_sb = const.tile([128, NDC, CW], bf16, tag="wo")
        nc.sync.dma_start(out=wq_sb, in_=wq_d[:, :].rearrange("(c p) n -> p c n", p=128))
        nc.gpsimd.dma_start(out=wk_sb, in_=wk_d[:, :].rearrange("(c p) n -> p c n", p=128))
        nc.scalar.dma_start(out=wv_sb, in_=wv_d[:, :].rearrange("(c p) n -> p c n", p=128))
        nc.scalar.dma_start(out=wo_sb, in_=wo_d[:, :].rearrange("(c p) n -> p c n", p=128))

        xt_sb = [pxt.tile([128, S], bf16, tag=f"xt{c}", name=f"xt{c}")
                 for c in range(NDC)]
        _qs = [nc.sync, nc.gpsimd, nc.scalar]
        # first column-halves land first: the prefix only needs keys 0:1024
        # (k0/q0 st0-1), so the exp stream starts ~20us earlier
        for half in range(2):
            for c in range(NDC):
                _qs[c % 3].dma_start(
                    out=xt_sb[c][:, half * S // 2:(half + 1) * S // 2],
                    in_=xt_d[c * 128:(c + 1) * 128,
                             half * S // 2:(half + 1) * S // 2])

        bqr_sb = const.tile([128, 2], f32, tag="bqr")
        bor_sb = const.tile([128, 2], f32, tag="bor")
        nc.sync.dma_start(out=bqr_sb, in_=bq_d[:, :])
        nc.sync.dma_start(out=bor_sb, in_=bo_d[:, :])
        bv0_sb = const.tile([64, HPC], f32, tag="bv0")
        nc.sync.dma_start(out=bv0_sb, in_=bv_d[:].rearrange("(h p) -> p h", p=64))
        bv_sb = const.tile([64, HPC], f32, tag="bv")
        nc.vector.tensor_copy(bv_sb, bv0_sb)  # pre-touch: keep deps DVE-local

        onesrow_sb = const.tile([1, QT], bf16, tag="onesrow")
        nc.vector.memset(onesrow_sb, 1.0)
        ones_sb = const.tile([64, 1], f32, tag="ones")
        nc.vector.memset(ones_sb, 1.0)
        ones2_sb = const.tile([2, 64], f32, tag="ones2")
        nc.vector.memset(ones2_sb, 1.0)

        qT_sb = big.tile([128, 2, S], bf16, tag="qT")   # pair t: head 2t rows 0:64
        kT_sb = big.tile([128, 2, S], bf16, tag="kT")
        v_sb = [big.tile([128, NKT, DH + 1], bf16, tag=f"v{h}", name=f"v{h}")
                for h in range(HPC)]
        for h in range(HPC):
            nc.vector.memset(v_sb[h][:, :, DH:DH + 1], 1.0)
        nrmg_sb = big.tile([128, NDC, S], bf16, tag="nrmg")

        # ---- prefix: pair-0 q/k for the first key half only, c-outer so
        # the PE paces with the arriving first-half x DMAs ----
        with tc.tile_pool(name="ppre", bufs=1, space="PSUM") as ppre:
            # dummy matmuls to lift the HAM clock gate off its cold state
            # while the x DMAs stream in
            scr = ppre.tile([64, QT], f32, tag="scr", name="scr")
            for _ in range(25):
                nc.tensor.matmul(scr, onesrow_sb[0:1, 0:64], onesrow_sb,
                                 start=True, stop=True)
            qps = [ppre.tile([128, QT], f32, tag=f"pq{st}", name=f"pq{st}")
                   for st in range(2)]
            kps = [ppre.tile([128, QT], f32, tag=f"pk{st}", name=f"pk{st}")
                   for st in range(2)]
            for c in range(NDC):
                for st in range(2):
                    nc.tensor.matmul(kps[st], wk_sb[:, c, 0:128],
                                     xt_sb[c][:, st * QT:(st + 1) * QT],
                                     start=(c == 0), stop=(c == NDC - 1))
                    nc.tensor.matmul(qps[st], wq_sb[:, c, 0:128],
                                     xt_sb[c][:, st * QT:(st + 1) * QT],
                                     start=(c == 0), stop=(c == NDC - 1))
            for st in range(2):
                nc.vector.tensor_copy(out=kT_sb[:, 0, st * QT:(st + 1) * QT],
                                      in_=kps[st])
                nc.vector.tensor_scalar(out=qT_sb[:, 0, st * QT:(st + 1) * QT],
                                        in0=qps[st], scalar1=bqr_sb[:, 0:1],
                                        scalar2=None, op0=ALU.add)

        with tc.tile_pool(name="pqk", bufs=2, space="PSUM") as pqk, \
             tc.tile_pool(name="psc", bufs=2, space="PSUM") as psc, \
             tc.tile_pool(name="pav", bufs=1, space="PSUM") as pav, \
             tc.tile_pool(name="pe", bufs=1) as pe_pool, \
             tc.tile_pool(name="pnrm", bufs=2) as pnrm, \
             tc.tile_pool(name="pd", bufs=1) as pd:

            bnst = [pd.tile([64, NQT, 6], f32, tag=f"bn{h}", name=f"bnst{h}")
                    for h in range(HPC)]
            vr_all = pd.tile([1, HPC], f32, tag="vr_all", name="vr_all")
            msc_all = pd.tile([1, 2 * HPC], f32, tag="msc", name="msc_all")
            stk_all = pd.tile([64, HPC, 3], f32, tag="stk", name="stk_all")
            eps_t = pd.tile([1, 1], f32, tag="eps", name="eps_t")
            nc.vector.memset(eps_t, EPS)

            vps = {}

            def v_half(pair, st, half):
                # V for heads (2*pair, 2*pair+1) at key block st, 4 c-chunks
                key = (pair, st)
                if half == 0:
                    vps[key] = pqk.tile([128, QT], f32, tag="qk",
                                        name=f"v{pair}{st}")
                ps = vps[key]
                for c in range(4 * half, 4 * half + 4):
                    nc.tensor.matmul(ps[:, 0:128],
                                     xt_sb[c][:, st * 128:(st + 1) * 128],
                                     wv_sb[:, c, pair * 128:(pair + 1) * 128],
                                     start=(c == 0), stop=(c == NDC - 1))
                if half == 1:
                    for i in range(2):
                        h = 2 * pair + i
                        nc.vector.tensor_copy(out=v_sb[h][:, st, 0:DH],
                                              in_=ps[:, i * DH:(i + 1) * DH])

            # deferred projections (pair-0 q/k second key-half, pair-1 q/k,
            # all V), emitted as small filler units inside the attention
            # loop so the PE never bursts long enough to starve the exps
            qkdps = {}

            def qkd_mm(t_, which, st, c):  # which: 0=q, 1=k
                w_sb = wq_sb if which == 0 else wk_sb
                key = (t_, which, st)
                if c == 0:
                    qkdps[key] = pqk.tile(
                        [128, QT], f32, tag="qk", name=f"qkd{t_}{which}{st}")
                nc.tensor.matmul(qkdps[key], w_sb[:, c, t_ * 128:(t_ + 1) * 128],
                                 xt_sb[c][:, st * QT:(st + 1) * QT],
                                 start=(c == 0), stop=(c == NDC - 1))

            def qkd_fin(t_, which, st):
                ps = qkdps[(t_, which, st)]
                if which == 0:
                    nc.vector.tensor_scalar(
                        out=qT_sb[:, t_, st * QT:(st + 1) * QT], in0=ps,
                        scalar1=bqr_sb[:, t_:t_ + 1], scalar2=None, op0=ALU.add)
                else:
                    nc.vector.tensor_copy(
                        out=kT_sb[:, t_, st * QT:(st + 1) * QT], in_=ps)

            fcount = {}
            fillers = []  # (cost, marker, fn)

            def add_qkd(t_, which, st, mark):
                for c in range(NDC):
                    fillers.append(
                        (512, None,
                         lambda t_=t_, w=which, st=st, c=c: qkd_mm(t_, w, st, c)))
                fillers.append(
                    (900, mark, lambda t_=t_, w=which, st=st: qkd_fin(t_, w, st)))

            def add_v(pair, st):
                fillers.append((600, None,
                                lambda pair=pair, st=st: v_half(pair, st, 0)))
                fillers.append((700, f"v{pair}",
                                lambda pair=pair, st=st: v_half(pair, st, 1)))

            add_v(0, 0)
            add_v(0, 1)
            add_qkd(0, 1, 2, "k0s2")
            add_v(0, 2)
            add_v(0, 3)
            add_qkd(0, 1, 3, "k0s3")
            add_v(0, 4)
            add_v(0, 5)
            add_qkd(0, 0, 2, "q0s2")
            add_v(0, 6)
            add_v(0, 7)
            add_qkd(0, 0, 3, "q0s3")
            for st in range(8, NKT):
                add_v(0, st)
            # pair-1 q/k ordered by first use so the t0->t1 boundary drains
            # gradually instead of as one ACT-starving burst
            add_qkd(1, 1, 0, "k1s0")
            add_qkd(1, 0, 0, "q1s0")
            add_qkd(1, 1, 1, "k1s1")
            add_qkd(1, 1, 2, "k1s2")
            add_qkd(1, 1, 3, "k1s3")
            add_qkd(1, 0, 1, "q1s1")
            add_qkd(1, 0, 2, "q1s2")
            add_qkd(1, 0, 3, "q1s3")
            for st in range(NKT):
                add_v(1, st)
            filler_spent = 0
            filler_budget = 0.0

            def pop_filler():
                nonlocal filler_spent
                cost, marker, fn = fillers.pop(0)
                fn()
                if marker is not None:
                    fcount[marker] = fcount.get(marker, 0) + 1
                filler_spent += cost
            zp_chunks = {}

            for t in range(2):
                h0, h1 = 2 * t, 2 * t + 1
                for qt in range(NQT):
                    if (t == 0 and qt >= 2) or t == 1:
                        while fillers and fcount.get(f"q{t}s{qt}", 0) < 1:
                            pop_filler()
                    e_sb = pe_pool.tile([128, NKT * 2 * QT], bf16,
                                        tag=f"e{qt % 2}", name=f"e{t}{qt}")
                    av = [pav.tile([DH + 1, QT], f32, tag=f"av{i}",
                                   name=f"av{t}{qt}{i}") for i in range(2)]

                    def emit_av(kt):
                        while fillers and fcount.get(f"v{t}", 0) <= kt:
                            pop_filler()
                        for i in range(2):
                            nc.tensor.matmul(
                                av[i], v_sb[2 * t + i][:, kt, :],
                                e_sb[:, kt * 1024 + i * QT:kt * 1024 + (i + 1) * QT],
                                start=(kt == 0), stop=(kt == NKT - 1))

                    for kt in range(NKT):
                        if t == 1 or kt // 4 >= 2:
                            while fillers and fcount.get(f"k{t}s{kt // 4}", 0) < 1:
                                pop_filler()
                        sps = psc.tile([128, 2 * QT], f32, tag="s",
                                       name=f"s{t}{qt}{kt}")
                        for o in range(2):
                            nc.tensor.matmul(
                                sps[:, o * QT:(o + 1) * QT],
                                kT_sb[64 * o:64 * (o + 1), t, kt * 128:(kt + 1) * 128],
                                qT_sb[64 * o:64 * (o + 1), t, qt * QT:(qt + 1) * QT],
                                start=True, stop=True)
                        nc.scalar.activation(
                            e_sb[:, kt * 1024:(kt + 1) * 1024], sps, AF.Exp)
                        filler_budget += 1050.0
                        while fillers and filler_spent < filler_budget:
                            pop_filler()
                        if kt >= 2:
                            emit_av(kt - 2)
                    emit_av(NKT - 2)
                    emit_av(NKT - 1)

                    # ---- per-(t,qt) softmax normalize + GN stats ----
                    last = (t == 1 and qt == NQT - 1)
                    # den rows out first (the reciprocal round-trip has DMA
                    # latency; both heads' trips overlap), then evacuate av
                    # PSUM so next qt's AVs aren't head-of-line blocked
                    # alternate DMA queues by qt parity so one qt's gather
                    # payload DMAs never head-of-line block the next qt's
                    # reciprocal round-trip
                    qa = nc.sync if qt % 2 == 0 else nc.gpsimd
                    qb = nc.gpsimd if qt % 2 == 0 else nc.sync
                    dnr = [pnrm.tile([1, QT], f32, tag=f"dn{i}",
                                     name=f"dn{t}{qt}{i}") for i in range(2)]
                    for i in range(2):
                        nc.vector.tensor_copy(dnr[i], av[i][DH:DH + 1, :])
                        qa.dma_start(out=rs_d[t, i, qt:qt + 1, :], in_=dnr[i])
                    zc = [pnrm.tile([DH, QT], f32, tag=f"zc{i}",
                                    name=f"zc{t}{qt}{i}") for i in range(2)]
                    nc.vector.tensor_copy(zc[0], av[0][0:DH, :])
                    nc.vector.tensor_copy(zc[1], av[1][0:DH, :])

                    if t == 0 or qt <= 1:
                        ci = 2 * t + qt // 2
                        cb = (qt % 2) * QT  # column base within gather chunk
                        if qt % 2 == 0:
                            zp = pnrm.tile([128, 2 * QT], bf16, tag="zp",
                                           name=f"zp{t}{qt}")
                            zp_chunks[ci] = zp
                        else:
                            zp = zp_chunks[ci]
                    else:
                        # t1 qt2/qt3 ship as single-qt chunks to shrink the
                        # exposed tail gather
                        cb = 0
                        zp = pnrm.tile([128, QT + SCC], bf16, tag="zps",
                                       name=f"zp{t}{qt}")
                    tmp1 = pnrm.tile([64, QT], bf16, tag="tmp1", name=f"tm{t}{qt}")
                    rbs = []
                    for i in range(2):
                        rb = pnrm.tile([64, QT], f32, tag=f"rb{i}", name=f"rb{t}{qt}{i}")
                        qb.dma_start(
                            out=rb,
                            in_=rs_d[t, i, qt:qt + 1, :].to_broadcast([64, QT]))
                        rbs.append(rb)
                    for i in range(2):
                        nc.vector.reciprocal_approx_fast(rbs[i], rbs[i])
                    nc.vector.tensor_mul(zp[0:64, cb:cb + QT], zc[0], rbs[0])
                    nc.vector.tensor_mul(tmp1, zc[1], rbs[1])
                    qb.dma_start(out=zp[64:128, cb:cb + QT], in_=tmp1)
                    nc.vector.bn_stats(out=bnst[h0][:, qt, :],
                                       in_=zp[0:64, cb:cb + QT])
                    nc.vector.bn_stats(out=bnst[h1][:, qt, :], in_=tmp1)

                    if qt == NQT - 1:
                        # pair-t GN stat fold (needs all 4 qt blocks)
                        for i, h in enumerate((h0, h1)):
                            mvh = pd.tile([64, 2], f32, tag="mv", bufs=2,
                                          name=f"mv{h}")
                            nc.vector.bn_aggr(out=mvh, in_=bnst[h])
                            stk = stk_all[:, h, :]
                            nc.vector.tensor_add(stk[:, 0:1], mvh[:, 0:1],
                                                 bv_sb[:, h:h + 1])
                            nc.vector.tensor_copy(stk[:, 1:2], mvh[:, 1:2])
                            nc.vector.tensor_mul(stk[:, 2:3], stk[:, 0:1],
                                                 stk[:, 0:1])

                    if last:
                        # ---- GN scalar tail: rstd via exp(-0.5*ln(var+eps)),
                        # same ACT table set as the softmax exps ----
                        stp = pav.tile([1, HPC * 3], f32, tag="av0", name="stp")
                        nc.tensor.matmul(stp, ones_sb,
                                         stk_all.rearrange("p h k -> p (h k)"),
                                         start=True, stop=True)
                        scg = pd.tile([1, HPC * 3], f32, tag="scg", name="scg")
                        nc.vector.tensor_copy(scg, stp)
                        e3 = pd.tile([1, HPC * 3], f32, tag="e3", name="e3")
                        nc.vector.tensor_scalar(out=e3, in0=scg, scalar1=1.0 / 64.0,
                                                scalar2=None, op0=ALU.mult)
                        e3r = e3.rearrange("p (h k) -> p h k", k=3)
                        m2 = pd.tile([1, HPC], f32, tag="m2", name="m2")
                        nc.vector.tensor_mul(m2, e3r[:, :, 0], e3r[:, :, 0])
                        nc.vector.tensor_add(vr_all, e3r[:, :, 1], e3r[:, :, 2])
                        nc.vector.tensor_tensor(out=vr_all, in0=vr_all, in1=m2,
                                                op=ALU.subtract)
                        sd_all = pd.tile([1, HPC], f32, tag="sd", name="sd_all")
                        nc.scalar.activation(sd_all, vr_all, AF.Sqrt, bias=eps_t)
                        rr = pd.tile([1, HPC], f32, tag="rr", name="rr")
                        nc.vector.reciprocal(rr, sd_all)
                        # natural payload order [M0..M3, r0..r3]; Phase E
                        # regroups by parity via a strided view
                        nc.vector.tensor_copy(msc_all[:, 0:HPC], e3r[:, :, 0])
                        nc.vector.tensor_copy(msc_all[:, HPC:2 * HPC], rr)
                        nc.vector.tensor_copy(out=zp[0:1, QT:QT + SCC],
                                              in_=msc_all[0:1, :].bitcast(bf16))
                        qa.dma_start(out=ag_in_last[:, :], in_=zp)
                        gather(ag_in_last[:], ag_out_last[:])
                    elif t == 1 and qt == 2:
                        qa.dma_start(out=ag_in4[:, :], in_=zp[:, 0:QT])
                        gather(ag_in4[:], ag_out4[:])
                    elif qt % 2 == 1:
                        qa.dma_start(out=ag_in[ci], in_=zp[:, 0:2 * QT])
                        gather(ag_in[ci], ag_out[ci])

            # nrmg loads after all chunks are in flight: early chunks are
            # long done (no queue-blocking wait); only the last chunks'
            # loads wait, inside their unavoidable gather window
            for ci in range(3):
                t2, qp = divmod(ci, 2)
                for g in range(4):
                    eng = nc.sync if g % 2 == 0 else nc.gpsimd
                    eng.dma_start(
                        out=nrmg_sb[:, 2 * g + t2, qp * 2 * QT:(qp + 1) * 2 * QT],
                        in_=ag_out[ci, g])
            for g in range(4):
                eng = nc.sync if g % 2 == 0 else nc.gpsimd
                eng2 = nc.gpsimd if g % 2 == 0 else nc.sync
                eng.dma_start(out=nrmg_sb[:, 2 * g + 1, 2 * QT:3 * QT],
                              in_=ag_out4[g])
                eng2.dma_start(out=nrmg_sb[:, 2 * g + 1, 3 * QT:4 * QT],
                               in_=ag_out_last[g, :, 0:QT])

        # ---- Phase E: fold GN affine into Wo, column-parallel out-proj ----
        with tc.tile_pool(name="pg", bufs=1) as pg, \
             tc.tile_pool(name="pf", bufs=4, space="PSUM") as pf, \
             tc.tile_pool(name="pystage", bufs=2) as pystage:
            # gathered scalars: [4 groups, 8 f32] as bitcast bf16 rows
            sc16 = pg.tile([1, 4, SCC], bf16, tag="sc16")
            nc.sync.dma_start(
                out=sc16,
                in_=ag_out_last[:, 0:1, QT:QT + SCC].rearrange("g p c -> p g c"))
            # PE sat idle through the gather wait and is HAM-cold; warm it
            # during the scalar fold so the out-proj runs at full clock
            scr2 = pf.tile([16, QT], f32, tag="scr2", bufs=1, name="scr2")
            for _ in range(16):
                nc.tensor.matmul(scr2, sc16[0:1, 0, :], onesrow_sb,
                                 start=True, stop=True)
            # [1, 4, 8] f32 per group: (M0..M3, r0..r3)
            scf = sc16[:, :, :].bitcast(f32)
            # per out-partition half o: values for chunks c=(g,t) are heads
            # 2t+o within each group -- strided parity regroup
            rstg = pg.tile([1, 2, 4, 2], f32, tag="rstg")
            mstg = pg.tile([1, 2, 4, 2], f32, tag="mstg")
            nc.vector.tensor_copy(
                out=mstg, in_=scf[:, :, 0:HPC].rearrange(
                    "p g (b o) -> p o g b", b=2, o=2))
            nc.vector.tensor_copy(
                out=rstg, in_=scf[:, :, HPC:2 * HPC].rearrange(
                    "p g (b o) -> p o g b", b=2, o=2))
            s2p = pf.tile([128, NDC], f32, tag="s2p", bufs=1, name="s2p")
            mcp = pf.tile([128, NDC], f32, tag="mcp", bufs=1, name="mcp")
            for o in range(2):
                nc.tensor.matmul(s2p[64 * o:64 * (o + 1), :], ones2_sb[0:1, :],
                                 rstg[:, o, :, :], start=True, stop=True)
                nc.tensor.matmul(mcp[64 * o:64 * (o + 1), :], ones2_sb[0:1, :],
                                 mstg[:, o, :, :], start=True, stop=True)
            s2c = pg.tile([128, NDC], f32, tag="s2c")
            nc.vector.tensor_copy(s2c, s2p)
            bvg = pg.tile([128, NDC], f32, tag="bvg")
            nc.sync.dma_start(out=bvg, in_=bvf_d[:].rearrange("(c p) -> p c", p=128))
            mcs = pg.tile([128, NDC], f32, tag="mcs")
            nc.vector.tensor_tensor(out=mcs, in0=mcp, in1=bvg, op=ALU.subtract)
            mvec = pg.tile([128, NDC], bf16, tag="mvec")
            nc.vector.tensor_mul(mvec, mcs, s2c)

            # wo_scaled[p, (c,n)] = r_head(p,c) * wo ; cstv[n] = sum_p M*r*wo
            wos = pg.tile([128, NDC, CW], bf16, tag="wos")
            for c in range(NDC):
                nc.vector.tensor_scalar(out=wos[:, c, :], in0=wo_sb[:, c, :],
                                        scalar1=s2c[:, c:c + 1], scalar2=None,
                                        op0=ALU.mult)
            cstv = pf.tile([128, 2], f32, tag="cst", bufs=1, name="cstv")
            for nt in range(2):
                for c in range(NDC):
                    nc.tensor.matmul(cstv[:, nt:nt + 1],
                                     wo_sb[:, c, nt * 128:(nt + 1) * 128],
                                     mvec[:, c:c + 1],
                                     start=(c == 0), stop=(c == NDC - 1))
            brows = pg.tile([128, 2], f32, tag="brows")
            nc.vector.tensor_tensor(out=brows, in0=bor_sb, in1=cstv,
                                    op=ALU.subtract)

            for nt in range(2):
                ystage = pystage.tile([128, S], f32, tag="ys", name=f"ys{nt}")
                yps = [pf.tile([128, QT], f32, tag="y", name=f"yp{nt}{st}")
                       for st in range(NQT)]
                # even chunks (pair 0, gathered early) first so the odd
                # chunks' late arrivals overlap real work
                for c in (0, 2, 4, 6, 1, 3, 5, 7):
                    for st in range(NQT):
                        nc.tensor.matmul(yps[st], wos[:, c, nt * 128:(nt + 1) * 128],
                                         nrmg_sb[:, c, st * QT:(st + 1) * QT],
                                         start=(c == 0), stop=(c == 7))
                for st in range(NQT):
                    # y bias (minus the folded GN mean term) rides the PSUM
                    # evacuation as a per-partition Identity bias
                    nc.scalar.activation(ystage[:, st * QT:(st + 1) * QT], yps[st],
                                         AF.Identity, bias=brows[:, nt:nt + 1])
                nc.sync.dma_start(out=y_d[nt, :, :], in_=ystage)

    nc.compile()
    return nc


def _get_nc():
    if "nc" not in _cache:
        _cache["nc"] = _build()
    return _cache["nc"]


def _host_prep(x, Wq, bq, Wk, bk, Wv, bv, Wo, bo, lq1, lk1, lq2, lk2, gn_w, gn_b):
    x = np.asarray(x, np.float32)
    lam = (np.exp((np.asarray(lq1) * np.asarray(lk1)).sum(-1))
           - np.exp((np.asarray(lq2) * np.asarray(lk2)).sum(-1)) + LAMBDA_INIT)
    qscale = (DH ** -0.5) * lam
    Wq_eff = (np.asarray(Wq).reshape(D, H, DH) * qscale[None, :, None]).reshape(D, D)
    bq_eff = (np.asarray(bq).reshape(H, DH) * qscale[:, None]).reshape(D)
    gw = np.asarray(gn_w).reshape(D)
    gb = np.asarray(gn_b).reshape(D)
    Wo_eff = np.asarray(Wo) * gw[:, None]
    bo_eff = np.asarray(bo) + gb @ np.asarray(Wo)

    # Gathered-row order (chunk (g,t), partition (o,dh) -> head 4g+2t+o) is
    # exactly the original row-major head order, so Wo_eff rows need no
    # permutation.  (bk is dropped: q.bk is constant along the softmax axis.)
    xT = np.ascontiguousarray(x.transpose(0, 2, 1))  # [B, D, S]
    bf = ml_dtypes.bfloat16

    in_maps = []
    for c in range(N_CORES):
        b, hg = c // 4, c % 4
        cs = slice(CW * hg, CW * (hg + 1))
        in_maps.append({
            "xt": np.ascontiguousarray(xT[b]).astype(bf),
            "wq": np.ascontiguousarray(Wq_eff[:, cs]).astype(bf),
            "wk": np.ascontiguousarray(np.asarray(Wk)[:, cs]).astype(bf),
            "wv": np.ascontiguousarray(np.asarray(Wv)[:, cs]).astype(bf),
            "wo": np.ascontiguousarray(Wo_eff[:, cs]).astype(bf),
            "bq": np.ascontiguousarray(bq_eff[cs].reshape(2, 128).T).astype(np.float32),
            "bv": np.ascontiguousarray(np.asarray(bv)[cs]).astype(np.float32),
            "bvf": np.ascontiguousarray(np.asarray(bv)).astype(np.float32),
            "bo": np.ascontiguousarray(bo_eff[cs].reshape(2, 128).T).astype(np.float32),
        })
    return in_maps


def _host_gather(outs):
    # core c=4b+hg produced output columns [256*hg, 256*(hg+1)) as [2,128,S]
    yT = np.empty((B, D, S), np.float32)
    for b in range(B):
        for hg in range(4):
            q = np.asarray(outs[4 * b + hg]["y"]).reshape(CW, S)
            yT[b, CW * hg:CW * (hg + 1), :] = q
    return np.ascontiguousarray(yT.transpose(0, 2, 1))


def kernel(x, Wq, bq, Wk, bk, Wv, bv, Wo, bo, lq1, lk1, lq2, lk2, gn_w, gn_b):
    from concourse.bass_utils import run_bass_kernel_spmd

    in_maps = _host_prep(x, Wq, bq, Wk, bk, Wv, bv, Wo, bo,
                         lq1, lk1, lq2, lk2, gn_w, gn_b)
    nc = _get_nc()
    res = run_bass_kernel_spmd(nc, in_maps, core_ids=list(range(N_CORES)))
    return _host_gather(res.results)


# revision 54
# speedup vs baseline: 1.1064x; 1.1064x over previous
"""Multi-head differential attention on 8 Trainium2 NeuronCores.

Sharding: core c -> batch c//4, head-group c%4 (4 of 16 heads).

v2 pipeline (vs baseline): the softmax exp stream on the Scalar engine is
the per-core floor (~128us), so everything is scheduled around keeping it
fed from t~14us onward:
  - prefix computes only q/k of head-pair 0; V and pair-1 q/k are emitted
    as PE filler inside pair-0's attention loop (the PE has slack while
    ACT chews exps).
  - attention inner loop is kt-pipelined: score pair (row-tiled 64x128
    auto-tiles) -> exp of [128,1024] PSUM tile -> e_sb (bf16, 16 kt tiles
    per (t,qt)) -> deferred AV (lag 2) accumulating z+denominator via the
    DH+1 ones-row.
  - softmax-normalize + GroupNorm stats + gather payload happen per
    (t,qt), and the z AllGather is split into 8 per-(t,qt) chunks that
    pipeline on the CC rings under the attention phase (the monolithic
    per-pair gathers were 2x57us, mostly exposed).
  - k-bias is dropped entirely (constant along the softmax axis), lambda
    and softmax scale are folded into Wq/bq, GroupNorm affine into Wo/bo
    on host; rstd uses exp(-0.5*ln(var+eps)) so the whole kernel needs
    one ACT table set (no mid-kernel sqrt table switch).
Each core then runs a column-parallel out-projection producing a
256-column slice of the output, assembled on host.
"""

import numpy as np
import ml_dtypes

B, S, D, H, DH = 2, 2048, 1024, 16, 64
HPC = 4            # heads per core
CW = HPC * DH      # attention columns per core (256)
EPS = 1e-5
LAMBDA_INIT = 0.8
N_CORES = 8
SCC = 16           # scalar payload columns (8 f32 as 16 bf16)
QT = 512           # q-block per (t, qt)
NQT = 4
NKT = 16
NDC = 8

_cache = {}


def _build(with_collective=True):
    from contextlib import ExitStack
    import concourse.bass as bass
    from concourse import bacc
    import concourse.tile as tile
    import concourse.mybir as mybir

    f32 = mybir.dt.float32
    bf16 = mybir.dt.bfloat16
    AF = mybir.ActivationFunctionType
    ALU = mybir.AluOpType

    nc = bacc.Bacc("TRN2", target_bir_lowering=False, debug=False,
                   num_devices=N_CORES)

    xt_d = nc.dram_tensor("xt", [D, S], bf16, kind="ExternalInput")
    wq_d = nc.dram_tensor("wq", [D, CW], bf16, kind="ExternalInput")
    wk_d = nc.dram_tensor("wk", [D, CW], bf16, kind="ExternalInput")
    wv_d = nc.dram_tensor("wv", [D, CW], bf16, kind="ExternalInput")
    wo_d = nc.dram_tensor("wo", [D, CW], bf16, kind="ExternalInput")
    # bq/bo in column layout [128, pair]: folded in as per-partition scalars
    bq_d = nc.dram_tensor("bq", [128, 2], f32, kind="ExternalInput")
    bv_d = nc.dram_tensor("bv", [CW], f32, kind="ExternalInput")
    bvf_d = nc.dram_tensor("bvf", [D], f32, kind="ExternalInput")
    bo_d = nc.dram_tensor("bo", [128, 2], f32, kind="ExternalInput")
    y_d = nc.dram_tensor("y", [2, 128, S], f32, kind="ExternalOutput")

    rs_d = nc.dram_tensor("rs_scratch", [2, 2, NQT, QT], f32)
    warm_in = nc.dram_tensor("warm_in", [1, 64], bf16)
    warm_out = nc.dram_tensor("warm_out", [4, 1, 64], bf16)
    # gather chunks: 3x 1024-col (t0qt01, t0qt23, t1qt01), then 512-col
    # t1qt2 and a final 512+SCC t1qt3 chunk carrying the GN scalars, so the
    # tail exposes only one small gather
    ag_in = nc.dram_tensor("ag_in", [3, 128, 2 * QT], bf16)
    ag_in4 = nc.dram_tensor("ag_in4", [128, QT], bf16)
    ag_in_last = nc.dram_tensor("ag_in_last", [128, QT + SCC], bf16)
    ag_out = nc.dram_tensor("ag_out", [3, 4, 128, 2 * QT], bf16)
    ag_out4 = nc.dram_tensor("ag_out4", [4, 128, QT], bf16)
    ag_out_last = nc.dram_tensor("ag_out_last", [4, 128, QT + SCC], bf16)

    GROUPS = [[0, 1, 2, 3], [4, 5, 6, 7]]

    def gather(in_ap, out_ap):
        if with_collective:
            nc.gpsimd.collective_compute(
                "AllGather", ALU.bypass, replica_groups=GROUPS,
                ins=[in_ap.opt()], outs=[out_ap.opt()])
        else:
            for g in range(4):
                nc.sync.dma_start(out=out_ap[g], in_=in_ap)

    with ExitStack() as ctx:
        tc = ctx.enter_context(tile.TileContext(nc))
        const = ctx.enter_context(tc.tile_pool(name="const", bufs=1))
        big = ctx.enter_context(tc.tile_pool(name="big", bufs=1))

        pxt = ctx.enter_context(tc.tile_pool(name="pxt", bufs=1))
        # warm-up collective input first: its gather absorbs the cross-core
        # rendezvous barrier, so it must not queue behind the bulk input DMAs
        wu_sb = pxt.tile([1, 64], bf16, tag="wu", name="wu")
        nc.vector.memset(wu_sb, 0.0)
        nc.sync.dma_start(out=warm_in[:, :], in_=wu_sb)

        # early dummy collective: absorbs the cross-core rendezvous barrier
        # under the prefix instead of delaying the first real gather
        gather(warm_in[:], warm_out[:])

        # ---- constants + x, spread over four DMA queues; q/k weights lead
        # so the prefix projections can pace with the arriving x chunks ----
        wq_sb = const.tile([128, NDC, CW], bf16, tag="wq")
        wk_sb = const.tile([128, NDC, CW], bf16, tag="wk")
        wv_sb = const.tile([128, NDC, CW], bf16, tag="wv")
        wo_sb = const.tile([128, NDC, CW], bf16, tag="wo")
        nc.sync.dma_start(out=wq_sb, in_=wq_d[:, :].rearrange("(c p) n -> p c n", p=128))
        nc.gpsimd.dma_start(out=wk_sb, in_=wk_d[:, :].rearrange("(c p) n -> p c n", p=128))
        nc.scalar.dma_start(out=wv_sb, in_=wv_d[:, :].rearrange("(c p) n -> p c n", p=128))
        nc.scalar.dma_start(out=wo_sb, in_=wo_d[:, :].rearrange("(c p) n -> p c n", p=128))

        xt_sb = [pxt.tile([128, S], bf16, tag=f"xt{c}", name=f"xt{c}")
                 for c in range(NDC)]
        _qs = [nc.sync, nc.gpsimd, nc.scalar]
        # first column-halves land first: the prefix only needs keys 0:1024
        # (k0/q0 st0-1), so the exp stream starts ~20us earlier
        for half in range(2):
            for c in range(NDC):
                _qs[c % 3].dma_start(
                    out=xt_sb[c][:, half * S // 2:(half + 1) * S // 2],
                    in_=xt_d[c * 128:(c + 1) * 128,
                             half * S // 2:(half + 1) * S // 2])

        bqr_sb = const.tile([128, 2], f32, tag="bqr")
        bor_sb = const.tile([128, 2], f32, tag="bor")
        nc.sync.dma_start(out=bqr_sb, in_=bq_d[:, :])
        nc.sync.dma_start(out=bor_sb, in_=bo_d[:, :])
        bv0_sb = const.tile([64, HPC], f32, tag="bv0")
        nc.sync.dma_start(out=bv0_sb, in_=bv_d[:].rearrange("(h p) -> p h", p=64))
        bv_sb = const.tile([64, HPC], f32, tag="bv")
        nc.vector.tensor_copy(bv_sb, bv0_sb)  # pre-touch: keep deps DVE-local

        onesrow_sb = const.tile([1, QT], bf16, tag="onesrow")
        nc.vector.memset(onesrow_sb, 1.0)
        ones_sb = const.tile([64, 1], f32, tag="ones")
        nc.vector.memset(ones_sb, 1.0)
        ones2_sb = const.tile([2, 64], f32, tag="ones2")
        nc.vector.memset(ones2_sb, 1.0)

        qT_sb = big.tile([128, 2, S], bf16, tag="qT")   # pair t: head 2t rows 0:64
        kT_sb = big.tile([128, 2, S], bf16, tag="kT")
        v_sb = [big.tile([128, NKT, DH + 1], bf16, tag=f"v{h}", name=f"v{h}")
                for h in range(HPC)]
        for h in range(HPC):
            nc.vector.memset(v_sb[h][:, :, DH:DH + 1], 1.0)
        nrmg_sb = big.tile([128, NDC, S], bf16, tag="nrmg")

        # ---- prefix: pair-0 q/k for the first key half only, c-outer so
        # the PE paces with the arriving first-half x DMAs ----
        with tc.tile_pool(name="ppre", bufs=1, space="PSUM") as ppre:
            # dummy matmuls to lift the HAM clock gate off its cold state
            # while the x DMAs stream in
            scr = ppre.tile([64, QT], f32, tag="scr", name="scr")
            for _ in range(25):
                nc.tensor.matmul(scr, onesrow_sb[0:1, 0:64], onesrow_sb,
                                 start=True, stop=True)
            qps = [ppre.tile([128, QT], f32, tag=f"pq{st}", name=f"pq{st}")
                   for st in range(2)]
            kps = [ppre.tile([128, QT], f32, tag=f"pk{st}", name=f"pk{st}")
                   for st in range(2)]
            for c in range(NDC):
                for st in range(2):
                    nc.tensor.matmul(kps[st], wk_sb[:, c, 0:128],
                                     xt_sb[c][:, st * QT:(st + 1) * QT],
                                     start=(c == 0), stop=(c == NDC - 1))
                    nc.tensor.matmul(qps[st], wq_sb[:, c, 0:128],
                                     xt_sb[c][:, st * QT:(st + 1) * QT],
                                     start=(c == 0), stop=(c == NDC - 1))
            for st in range(2):
                nc.vector.tensor_copy(out=kT_sb[:, 0, st * QT:(st + 1) * QT],
                                      in_=kps[st])
                nc.vector.tensor_scalar(out=qT_sb[:, 0, st * QT:(st + 1) * QT],
                                        in0=qps[st], scalar1=bqr_sb[:, 0:1],
                                        scalar2=None, op0=ALU.add)

        with tc.tile_pool(name="pqk", bufs=2, space="PSUM") as pqk, \
             tc.tile_pool(name="psc", bufs=2, space="PSUM") as psc, \
             tc.tile_pool(name="pav", bufs=1, space="PSUM") as pav, \
             tc.tile_pool(name="pe", bufs=1) as pe_pool, \
             tc.tile_pool(name="pnrm", bufs=2) as pnrm, \
             tc.tile_pool(name="pd", bufs=1) as pd:

            bnst = [pd.tile([64, NQT, 6], f32, tag=f"bn{h}", name=f"bnst{h}")
                    for h in range(HPC)]
            vr_all = pd.tile([1, HPC], f32, tag="vr_all", name="vr_all")
            msc_all = pd.tile([1, 2 * HPC], f32, tag="msc", name="msc_all")
            stk_all = pd.tile([64, HPC, 3], f32, tag="stk", name="stk_all")
            eps_t = pd.tile([1, 1], f32, tag="eps", name="eps_t")
            nc.vector.memset(eps_t, EPS)

            vps = {}

            def v_half(pair, st, half):
                # V for heads (2*pair, 2*pair+1) at key block st, 4 c-chunks
                key = (pair, st)
                if half == 0:
                    vps[key] = pqk.tile([128, QT], f32, tag="qk",
                                        name=f"v{pair}{st}")
                ps = vps[key]
                for c in range(4 * half, 4 * half + 4):
                    nc.tensor.matmul(ps[:, 0:128],
                                     xt_sb[c][:, st * 128:(st + 1) * 128],
                                     wv_sb[:, c, pair * 128:(pair + 1) * 128],
                                     start=(c == 0), stop=(c == NDC - 1))
                if half == 1:
                    for i in range(2):
                        h = 2 * pair + i
                        nc.vector.tensor_copy(out=v_sb[h][:, st, 0:DH],
                                              in_=ps[:, i * DH:(i + 1) * DH])

            # deferred projections (pair-0 q/k second key-half, pair-1 q/k,
            # all V), emitted as small filler units inside the attention
            # loop so the PE never bursts long enough to starve the exps
            qkdps = {}

            def qkd_mm(t_, which, st, c):  # which: 0=q, 1=k
                w_sb = wq_sb if which == 0 else wk_sb
                key = (t_, which, st)
                if c == 0:
                    qkdps[key] = pqk.tile(
                        [128, QT], f32, tag="qk", name=f"qkd{t_}{which}{st}")
                nc.tensor.matmul(qkdps[key], w_sb[:, c, t_ * 128:(t_ + 1) * 128],
                                 xt_sb[c][:, st * QT:(st + 1) * QT],
                                 start=(c == 0), stop=(c == NDC - 1))

            def qkd_fin(t_, which, st):
                ps = qkdps[(t_, which, st)]
                if which == 0:
                    nc.vector.tensor_scalar(
                        out=qT_sb[:, t_, st * QT:(st + 1) * QT], in0=ps,
                        scalar1=bqr_sb[:, t_:t_ + 1], scalar2=None, op0=ALU.add)
                else:
                    nc.vector.tensor_copy(
                        out=kT_sb[:, t_, st * QT:(st + 1) * QT], in_=ps)

            fcount = {}
            fillers = []  # (cost, marker, fn)

            def add_qkd(t_, which, st, mark):
                for c in range(NDC):
                    fillers.append(
                        (512, None,
                         lambda t_=t_, w=which, st=st, c=c: qkd_mm(t_, w, st, c)))
                fillers.append(
                    (900, mark, lambda t_=t_, w=which, st=st: qkd_fin(t_, w, st)))

            def add_v(pair, st):
                fillers.append((600, None,
                                lambda pair=pair, st=st: v_half(pair, st, 0)))
                fillers.append((700, f"v{pair}",
                                lambda pair=pair, st=st: v_half(pair, st, 1)))

            add_v(0, 0)
            add_v(0, 1)
            add_qkd(0, 1, 2, "k0s2")
            add_v(0, 2)
            add_v(0, 3)
            add_qkd(0, 1, 3, "k0s3")
            add_v(0, 4)
            add_v(0, 5)
            add_qkd(0, 0, 2, "q0s2")
            add_v(0, 6)
            add_v(0, 7)
            add_qkd(0, 0, 3, "q0s3")
            for st in range(8, NKT):
                add_v(0, st)
            for which in (1, 0):
                for st in range(NQT):
                    add_qkd(1, which, st, "qk1")
            for st in range(NKT):
                add_v(1, st)
            filler_spent = 0
            filler_budget = 0.0

            def pop_filler():
                nonlocal filler_spent
                cost, marker, fn = fillers.pop(0)
                fn()
                if marker is not None:
                    fcount[marker] = fcount.get(marker, 0) + 1
                filler_spent += cost
            zp_chunks = {}

            for t in range(2):
                h0, h1 = 2 * t, 2 * t + 1
                if t == 1:
                    # pair-1 q/k must be complete before its first score
                    while fillers and fcount.get("qk1", 0) < 8:
                        pop_filler()
                for qt in range(NQT):
                    if t == 0 and qt >= 2:
                        while fillers and fcount.get(f"q0s{qt}", 0) < 1:
                            pop_filler()
                    e_sb = pe_pool.tile([128, NKT * 2 * QT], bf16,
                                        tag=f"e{qt % 2}", name=f"e{t}{qt}")
                    av = [pav.tile([DH + 1, QT], f32, tag=f"av{i}",
                                   name=f"av{t}{qt}{i}") for i in range(2)]

                    def emit_av(kt):
                        while fillers and fcount.get(f"v{t}", 0) <= kt:
                            pop_filler()
                        for i in range(2):
                            nc.tensor.matmul(
                                av[i], v_sb[2 * t + i][:, kt, :],
                                e_sb[:, kt * 1024 + i * QT:kt * 1024 + (i + 1) * QT],
                                start=(kt == 0), stop=(kt == NKT - 1))

                    for kt in range(NKT):
                        if t == 0 and kt // 4 >= 2:
                            while fillers and fcount.get(f"k0s{kt // 4}", 0) < 1:
                                pop_filler()
                        sps = psc.tile([128, 2 * QT], f32, tag="s",
                                       name=f"s{t}{qt}{kt}")
                        for o in range(2):
                            nc.tensor.matmul(
                                sps[:, o * QT:(o + 1) * QT],
                                kT_sb[64 * o:64 * (o + 1), t, kt * 128:(kt + 1) * 128],
                                qT_sb[64 * o:64 * (o + 1), t, qt * QT:(qt + 1) * QT],
                                start=True, stop=True)
                        nc.scalar.activation(
                            e_sb[:, kt * 1024:(kt + 1) * 1024], sps, AF.Exp)
                        filler_budget += 1050.0
                        while fillers and filler_spent < filler_budget:
                            pop_filler()
                        if kt >= 2:
                            emit_av(kt - 2)
                    emit_av(NKT - 2)
                    emit_av(NKT - 1)

                    # ---- per-(t,qt) softmax normalize + GN stats ----
                    last = (t == 1 and qt == NQT - 1)
                    # den rows out first (the reciprocal round-trip has DMA
                    # latency; both heads' trips overlap), then evacuate av
                    # PSUM so next qt's AVs aren't head-of-line blocked
                    # alternate DMA queues by qt parity so one qt's gather
                    # payload DMAs never head-of-line block the next qt's
                    # reciprocal round-trip
                    qa = nc.sync if qt % 2 == 0 else nc.gpsimd
                    qb = nc.gpsimd if qt % 2 == 0 else nc.sync
                    dnr = [pnrm.tile([1, QT], f32, tag=f"dn{i}",
                                     name=f"dn{t}{qt}{i}") for i in range(2)]
                    for i in range(2):
                        nc.vector.tensor_copy(dnr[i], av[i][DH:DH + 1, :])
                        qa.dma_start(out=rs_d[t, i, qt:qt + 1, :], in_=dnr[i])
                    zc = [pnrm.tile([DH, QT], f32, tag=f"zc{i}",
                                    name=f"zc{t}{qt}{i}") for i in range(2)]
                    nc.vector.tensor_copy(zc[0], av[0][0:DH, :])
                    nc.vector.tensor_copy(zc[1], av[1][0:DH, :])

                    if t == 0 or qt <= 1:
                        ci = 2 * t + qt // 2
                        cb = (qt % 2) * QT  # column base within gather chunk
                        if qt % 2 == 0:
                            zp = pnrm.tile([128, 2 * QT], bf16, tag="zp",
                                           name=f"zp{t}{qt}")
                            zp_chunks[ci] = zp
                        else:
                            zp = zp_chunks[ci]
                    else:
                        # t1 qt2/qt3 ship as single-qt chunks to shrink the
                        # exposed tail gather
                        cb = 0
                        zp = pnrm.tile([128, QT + SCC], bf16, tag="zps",
                                       name=f"zp{t}{qt}")
                    tmp1 = pnrm.tile([64, QT], bf16, tag="tmp1", name=f"tm{t}{qt}")
                    rbs = []
                    for i in range(2):
                        rb = pnrm.tile([64, QT], f32, tag=f"rb{i}", name=f"rb{t}{qt}{i}")
                        qb.dma_start(
                            out=rb,
                            in_=rs_d[t, i, qt:qt + 1, :].to_broadcast([64, QT]))
                        rbs.append(rb)
                    for i in range(2):
                        nc.vector.reciprocal_approx_fast(rbs[i], rbs[i])
                    nc.vector.tensor_mul(zp[0:64, cb:cb + QT], zc[0], rbs[0])
                    nc.vector.tensor_mul(tmp1, zc[1], rbs[1])
                    qb.dma_start(out=zp[64:128, cb:cb + QT], in_=tmp1)
                    nc.vector.bn_stats(out=bnst[h0][:, qt, :],
                                       in_=zp[0:64, cb:cb + QT])
                    nc.vector.bn_stats(out=bnst[h1][:, qt, :], in_=tmp1)

                    if qt == NQT - 1:
                        # pair-t GN stat fold (needs all 4 qt blocks)
                        for i, h in enumerate((h0, h1)):
                            mvh = pd.tile([64, 2], f32, tag="mv", bufs=2,
                                          name=f"mv{h}")
                            nc.vector.bn_aggr(out=mvh, in_=bnst[h])
                            stk = stk_all[:, h, :]
                            nc.vector.tensor_add(stk[:, 0:1], mvh[:, 0:1],
                                                 bv_sb[:, h:h + 1])
                            nc.vector.tensor_copy(stk[:, 1:2], mvh[:, 1:2])
                            nc.vector.tensor_mul(stk[:, 2:3], stk[:, 0:1],
                                                 stk[:, 0:1])

                    if last:
                        # ---- GN scalar tail: rstd via exp(-0.5*ln(var+eps)),
                        # same ACT table set as the softmax exps ----
                        stp = pav.tile([1, HPC * 3], f32, tag="av0", name="stp")
                        nc.tensor.matmul(stp, ones_sb,
                                         stk_all.rearrange("p h k -> p (h k)"),
                                         start=True, stop=True)
                        scg = pd.tile([1, HPC * 3], f32, tag="scg", name="scg")
                        nc.vector.tensor_copy(scg, stp)
                        e3 = pd.tile([1, HPC * 3], f32, tag="e3", name="e3")
                        nc.vector.tensor_scalar(out=e3, in0=scg, scalar1=1.0 / 64.0,
                                                scalar2=None, op0=ALU.mult)
                        e3r = e3.rearrange("p (h k) -> p h k", k=3)
                        m2 = pd.tile([1, HPC], f32, tag="m2", name="m2")
                        nc.vector.tensor_mul(m2, e3r[:, :, 0], e3r[:, :, 0])
                        nc.vector.tensor_add(vr_all, e3r[:, :, 1], e3r[:, :, 2])
                        nc.vector.tensor_tensor(out=vr_all, in0=vr_all, in1=m2,
                                                op=ALU.subtract)
                        sd_all = pd.tile([1, HPC], f32, tag="sd", name="sd_all")
                        nc.scalar.activation(sd_all, vr_all, AF.Sqrt, bias=eps_t)
                        rr = pd.tile([1, HPC], f32, tag="rr", name="rr")
                        nc.vector.reciprocal(rr, sd_all)
                        # natural payload order [M0..M3, r0..r3]; Phase E
                        # regroups by parity via a strided view
                        nc.vector.tensor_copy(msc_all[:, 0:HPC], e3r[:, :, 0])
                        nc.vector.tensor_copy(msc_all[:, HPC:2 * HPC], rr)
                        nc.vector.tensor_copy(out=zp[0:1, QT:QT + SCC],
                                              in_=msc_all[0:1, :].bitcast(bf16))
                        qa.dma_start(out=ag_in_last[:, :], in_=zp)
                        gather(ag_in_last[:], ag_out_last[:])
                    elif t == 1 and qt == 2:
                        qa.dma_start(out=ag_in4[:, :], in_=zp[:, 0:QT])
                        gather(ag_in4[:], ag_out4[:])
                    elif qt % 2 == 1:
                        qa.dma_start(out=ag_in[ci], in_=zp[:, 0:2 * QT])
                        gather(ag_in[ci], ag_out[ci])

            # nrmg loads after all chunks are in flight: early chunks are
            # long done (no queue-blocking wait); only the last chunks'
            # loads wait, inside their unavoidable gather window
            for ci in range(3):
                t2, qp = divmod(ci, 2)
                for g in range(4):
                    eng = nc.sync if g % 2 == 0 else nc.gpsimd
                    eng.dma_start(
                        out=nrmg_sb[:, 2 * g + t2, qp * 2 * QT:(qp + 1) * 2 * QT],
                        in_=ag_out[ci, g])
            for g in range(4):
                eng = nc.sync if g % 2 == 0 else nc.gpsimd
                eng2 = nc.gpsimd if g % 2 == 0 else nc.sync
                eng.dma_start(out=nrmg_sb[:, 2 * g + 1, 2 * QT:3 * QT],
                              in_=ag_out4[g])
                eng2.dma_start(out=nrmg_sb[:, 2 * g + 1, 3 * QT:4 * QT],
                               in_=ag_out_last[g, :, 0:QT])

        # ---- Phase E: fold GN affine into Wo, column-parallel out-proj ----
        with tc.tile_pool(name="pg", bufs=1) as pg, \
             tc.tile_pool(name="pf", bufs=4, space="PSUM") as pf, \
             tc.tile_pool(name="pystage", bufs=2) as pystage:
            # gathered scalars: [4 groups, 8 f32] as bitcast bf16 rows
            sc16 = pg.tile([1, 4, SCC], bf16, tag="sc16")
            nc.sync.dma_start(
                out=sc16,
                in_=ag_out_last[:, 0:1, QT:QT + SCC].rearrange("g p c -> p g c"))
            # PE sat idle through the gather wait and is HAM-cold; warm it
            # during the scalar fold so the out-proj runs at full clock
            scr2 = pf.tile([16, QT], f32, tag="scr2", bufs=1, name="scr2")
            for _ in range(16):
                nc.tensor.matmul(scr2, sc16[0:1, 0, :], onesrow_sb,
                                 start=True, stop=True)
            # [1, 4, 8] f32 per group: (M0..M3, r0..r3)
            scf = sc16[:, :, :].bitcast(f32)
            # per out-partition half o: values for chunks c=(g,t) are heads
            # 2t+o within each group -- strided parity regroup
            rstg = pg.tile([1, 2, 4, 2], f32, tag="rstg")
            mstg = pg.tile([1, 2, 4, 2], f32, tag="mstg")
            nc.vector.tensor_copy(
                out=mstg, in_=scf[:, :, 0:HPC].rearrange(
                    "p g (b o) -> p o g b", b=2, o=2))
            nc.vector.tensor_copy(
                out=rstg, in_=scf[:, :, HPC:2 * HPC].rearrange(
                    "p g (b o) -> p o g b", b=2, o=2))
            s2p = pf.tile([128, NDC], f32, tag="s2p", bufs=1, name="s2p")
            mcp = pf.tile([128, NDC], f32, tag="mcp", bufs=1, name="mcp")
            for o in range(2):
                nc.tensor.matmul(s2p[64 * o:64 * (o + 1), :], ones2_sb[0:1, :],
                                 rstg[:, o, :, :], start=True, stop=True)
                nc.tensor.matmul(mcp[64 * o:64 * (o + 1), :], ones2_sb[0:1, :],
                                 mstg[:, o, :, :], start=True, stop=True)
            s2c = pg.tile([128, NDC], f32, tag="s2c")
            nc.vector.tensor_copy(s2c, s2p)
            bvg = pg.tile([128, NDC], f32, tag="bvg")
            nc.sync.dma_start(out=bvg, in_=bvf_d[:].rearrange("(c p) -> p c", p=128))
            mcs = pg.tile([128, NDC], f32, tag="mcs")
            nc.vector.tensor_tensor(out=mcs, in0=mcp, in1=bvg, op=ALU.subtract)
            mvec = pg.tile([128, NDC], bf16, tag="mvec")
            nc.vector.tensor_mul(mvec, mcs, s2c)

            # wo_scaled[p, (c,n)] = r_head(p,c) * wo ; cstv[n] = sum_p M*r*wo
            wos = pg.tile([128, NDC, CW], bf16, tag="wos")
            for c in range(NDC):
                nc.vector.tensor_scalar(out=wos[:, c, :], in0=wo_sb[:, c, :],
                                        scalar1=s2c[:, c:c + 1], scalar2=None,
                                        op0=ALU.mult)
            cstv = pf.tile([128, 2], f32, tag="cst", bufs=1, name="cstv")
            for nt in range(2):
                for c in range(NDC):
                    nc.tensor.matmul(cstv[:, nt:nt + 1],
                                     wo_sb[:, c, nt * 128:(nt + 1) * 128],
                                     mvec[:, c:c + 1],
                                     start=(c == 0), stop=(c == NDC - 1))
            brows = pg.tile([128, 2], f32, tag="brows")
            nc.vector.tensor_tensor(out=brows, in0=bor_sb, in1=cstv,
                                    op=ALU.subtract)

            for nt in range(2):
                ystage = pystage.tile([128, S], f32, tag="ys", name=f"ys{nt}")
                yps = [pf.tile([128, QT], f32, tag="y", name=f"yp{nt}{st}")
                       for st in range(NQT)]
                # even chunks (pair 0, gathered early) first so the odd
                # chunks' late arrivals overlap real work
                for c in (0, 2, 4, 6, 1, 3, 5, 7):
                    for st in range(NQT):
                        nc.tensor.matmul(yps[st], wos[:, c, nt * 128:(nt + 1) * 128],
                                         nrmg_sb[:, c, st * QT:(st + 1) * QT],
                                         start=(c == 0), stop=(c == 7))
                for st in range(NQT):
                    # y bias (minus the folded GN mean term) rides the PSUM
                    # evacuation as a per-partition Identity bias
                    nc.scalar.activation(ystage[:, st * QT:(st + 1) * QT], yps[st],
                                         AF.Identity, bias=brows[:, nt:nt + 1])
                nc.sync.dma_start(out=y_d[nt, :, :], in_=ystage)

    nc.compile()
    return nc


def _get_nc():
    if "nc" not in _cache:
        _cache["nc"] = _build()
    return _cache["nc"]


def _host_prep(x, Wq, bq, Wk, bk, Wv, bv, Wo, bo, lq1, lk1, lq2, lk2, gn_w, gn_b):
    x = np.asarray(x, np.float32)
    lam = (np.exp((np.asarray(lq1) * np.asarray(lk1)).sum(-1))
           - np.exp((np.asarray(lq2) * np.asarray(lk2)).sum(-1)) + LAMBDA_INIT)
    qscale = (DH ** -0.5) * lam
    Wq_eff = (np.asarray(Wq).reshape(D, H, DH) * qscale[None, :, None]).reshape(D, D)
    bq_eff = (np.asarray(bq).reshape(H, DH) * qscale[:, None]).reshape(D)
    gw = np.asarray(gn_w).reshape(D)
    gb = np.asarray(gn_b).reshape(D)
    Wo_eff = np.asarray(Wo) * gw[:, None]
    bo_eff = np.asarray(bo) + gb @ np.asarray(Wo)

    # Gathered-row order (chunk (g,t), partition (o,dh) -> head 4g+2t+o) is
    # exactly the original row-major head order, so Wo_eff rows need no
    # permutation.  (bk is dropped: q.bk is constant along the softmax axis.)
    xT = np.ascontiguousarray(x.transpose(0, 2, 1))  # [B, D, S]
    bf = ml_dtypes.bfloat16

    in_maps = []
    for c in range(N_CORES):
        b, hg = c // 4, c % 4
        cs = slice(CW * hg, CW * (hg + 1))
        in_maps.append({
            "xt": np.ascontiguousarray(xT[b]).astype(bf),
            "wq": np.ascontiguousarray(Wq_eff[:, cs]).astype(bf),
            "wk": np.ascontiguousarray(np.asarray(Wk)[:, cs]).astype(bf),
            "wv": np.ascontiguousarray(np.asarray(Wv)[:, cs]).astype(bf),
            "wo": np.ascontiguousarray(Wo_eff[:, cs]).astype(bf),
            "bq": np.ascontiguousarray(bq_eff[cs].reshape(2, 128).T).astype(np.float32),
            "bv": np.ascontiguousarray(np.asarray(bv)[cs]).astype(np.float32),
            "bvf": np.ascontiguousarray(np.asarray(bv)).astype(np.float32),
            "bo": np.ascontiguousarray(bo_eff[cs].reshape(2, 128).T).astype(np.float32),
        })
    return in_maps


def _host_gather(outs):
    # core c=4b+hg produced output columns [256*hg, 256*(hg+1)) as [2,128,S]
    yT = np.empty((B, D, S), np.float32)
    for b in range(B):
        for hg in range(4):
            q = np.asarray(outs[4 * b + hg]["y"]).reshape(CW, S)
            yT[b, CW * hg:CW * (hg + 1), :] = q
    return np.ascontiguousarray(yT.transpose(0, 2, 1))


def kernel(x, Wq, bq, Wk, bk, Wv, bv, Wo, bo, lq1, lk1, lq2, lk2, gn_w, gn_b):
    from concourse.bass_utils import run_bass_kernel_spmd

    in_maps = _host_prep(x, Wq, bq, Wk, bk, Wv, bv, Wo, bo,
                         lq1, lk1, lq2, lk2, gn_w, gn_b)
    nc = _get_nc()
    res = run_bass_kernel_spmd(nc, in_maps, core_ids=list(range(N_CORES)))
    return _host_gather(res.results)
